# revision 1
# baseline (speedup 1.0000x reference)
# Multi-head attention layer on 8 TRN2 NeuronCores (SPMD, no collectives).
#
# Problem: B=4, N=2048, D=512, H=8 heads (DK=64).
#   out = softmax((q@Wq+bq)(k@Wk+bk)^T / 8) (v@Wv+bv) @ Wo + bo   per (batch, head)
#
# Sharding: core c handles batch b=c//2 and query-row half c%2 (1024 rows).
# K/V projections are recomputed by both cores of a pair (cheap) so there is
# no cross-core communication at all.
#
# Per-core dataflow (all layouts chosen so NO on-chip transposes are needed;
# the host pre-transposes inputs to (channel, token) layout and casts bf16):
#   K^T(d,k)  = Wk-chunks.T @ kT          (PE), +bias via DVE copy
#   Q^T(d,q)  = Wq-chunks.T @ qT          (PE), +bias via DVE copy
#   V(k,d)    = vT-chunks.T @ Wv          (PE), +bias via rank-1 matmul,
#               stored per-head as [V(64)|1|pad] blocks (pitch 66)
#   S^T(k,q)  = K^T_tile.T @ Q^T  per head  (PE, contraction d=64)
#   P^T       = exp(S^T/8)                (ACT, scale folded into activation)
#   ctx^T(d,q)= [V|1]-block.T @ P^T       (PE, accumulated over k; the ones
#               column makes the softmax denominator land in row 64)
#   norm      = ctx^T * broadcast(1/rowsum)  (DVE recip + DRAM-bounce DMA bcast)
#   out(n,d)  = ctxn-chunks.T @ Wo + bo   (PE, bias via rank-1 matmul)
from contextlib import ExitStack

import numpy as np
import ml_dtypes

import concourse.bass as bass
import concourse.mybir as mybir
import concourse.tile as tile
from concourse import bacc
from concourse.bass_utils import run_bass_kernel_spmd

BF16 = mybir.dt.bfloat16
F32 = mybir.dt.float32
Exp = mybir.ActivationFunctionType.Exp

B, N, D, H = 4, 2048, 512, 8
DK = D // H          # 64
NQ = N // 2          # 1024 query rows per core
HP = 66              # per-head pitch in V_s: [one, V(64), one]
NKT = N // 128       # 16 k tiles


def build_nc():
    nc = bacc.Bacc("TRN2", target_bir_lowering=False)

    qT = nc.dram_tensor("qT", (D, NQ), BF16, kind="ExternalInput")
    kT = nc.dram_tensor("kT", (D, N), BF16, kind="ExternalInput")
    vT = nc.dram_tensor("vT", (D, N), BF16, kind="ExternalInput")
    wq = nc.dram_tensor("wq", (D, D), BF16, kind="ExternalInput")
    wk = nc.dram_tensor("wk", (D, D), BF16, kind="ExternalInput")
    wv = nc.dram_tensor("wv", (D, D), BF16, kind="ExternalInput")
    wo = nc.dram_tensor("wo", (D, D), BF16, kind="ExternalInput")
    bq = nc.dram_tensor("bq", (D, 1), F32, kind="ExternalInput")
    bk = nc.dram_tensor("bk", (D, 1), F32, kind="ExternalInput")
    bv = nc.dram_tensor("bv", (1, D), BF16, kind="ExternalInput")
    bo = nc.dram_tensor("bo", (1, D), BF16, kind="ExternalInput")
    out = nc.dram_tensor("out", (NQ, D), F32, kind="ExternalOutput")

    with tile.TileContext(nc) as tc:
        with ExitStack() as ctx:
            emit(ctx, tc, qT, kT, vT, wq, wk, wv, wo, bq, bk, bv, bo, out)
    nc.compile()
    return nc


def emit(ctx, tc, qT, kT, vT, wq, wk, wv, wo, bq, bk, bv, bo, out, dbg=None):
    nc = tc.nc
    consts = ctx.enter_context(tc.tile_pool(name="consts", bufs=1))
    p_pool = ctx.enter_context(tc.tile_pool(name="p_pool", bufs=6))
    post = ctx.enter_context(tc.tile_pool(name="post", bufs=2))
    outs = ctx.enter_context(tc.tile_pool(name="outs", bufs=3))
    s_pool = ctx.enter_context(tc.tile_pool(name="s_pool", bufs=2, space="PSUM"))
    c_pool = ctx.enter_context(tc.tile_pool(name="c_pool", bufs=2, space="PSUM"))
    dram = ctx.enter_context(tc.tile_pool(name="dram", bufs=2, space="DRAM"))

    # ---- constants / inputs -------------------------------------------------
    ones = consts.tile([1, D], BF16)
    nc.vector.memset(ones, 1.0)

    def load(name, shape, dt_, src_ap):
        t = consts.tile(shape, dt_, name=name)
        nc.sync.dma_start(out=t, in_=src_ap)
        return t

    wq_s = load("wq_s", [128, 4, D], BF16, wq[:].rearrange("(c p) d -> p c d", p=128))
    wk_s = load("wk_s", [128, 4, D], BF16, wk[:].rearrange("(c p) d -> p c d", p=128))
    wv_s = load("wv_s", [128, 4, D], BF16, wv[:].rearrange("(c p) d -> p c d", p=128))
    wo_s = load("wo_s", [128, 4, D], BF16, wo[:].rearrange("(c p) d -> p c d", p=128))
    bq_s = load("bq_s", [128, 4, 1], F32, bq[:].rearrange("(c p) o -> p c o", p=128))
    bk_s = load("bk_s", [128, 4, 1], F32, bk[:].rearrange("(c p) o -> p c o", p=128))
    bv_s = load("bv_s", [1, D], BF16, bv[:])
    bo_s = load("bo_s", [1, D], BF16, bo[:])
    qT_s = load("qT_s", [128, 4, NQ], BF16, qT[:].rearrange("(c p) n -> p c n", p=128))
    kT_s = load("kT_s", [128, 4, N], BF16, kT[:].rearrange("(c p) n -> p c n", p=128))
    vT_s = load("vT_s", [128, 4, N], BF16, vT[:].rearrange("(c p) n -> p c n", p=128))

    KT_s = consts.tile([128, 4, N], BF16)     # K^T, d on partitions
    QT_s = consts.tile([128, 4, NQ], BF16)    # Q^T, d on partitions
    V_s = consts.tile([128, NKT, H, HP], BF16)  # V, k on partitions, [V(64)|1|pad]
    ctxn_s = consts.tile([128, 4, NQ], BF16)  # normalized ctx^T, dmid on partitions

    nc.vector.memset(V_s[:, :, :, 64:65], 1.0)

    # ---- projections --------------------------------------------------------
    def emit_kproj(dt):
        for kh in range(2):
            st = s_pool.tile([128, 1024], F32, tag="s", name="st_k")
            for kc in range(2):
                for cc in range(4):
                    nc.tensor.matmul(
                        st[:, kc * 512:(kc + 1) * 512],
                        lhsT=wk_s[:, cc, dt * 128:(dt + 1) * 128],
                        rhs=kT_s[:, cc, kh * 1024 + kc * 512: kh * 1024 + (kc + 1) * 512],
                        start=(cc == 0), stop=(cc == 3))
            nc.vector.tensor_scalar_add(
                KT_s[:, dt, kh * 1024:(kh + 1) * 1024], st, bk_s[:, dt, :])

    def emit_qproj(dt):
        st = c_pool.tile([128, 1024], F32, tag="c", name="st_q")
        for qc in range(2):
            for cc in range(4):
                nc.tensor.matmul(
                    st[:, qc * 512:(qc + 1) * 512],
                    lhsT=wq_s[:, cc, dt * 128:(dt + 1) * 128],
                    rhs=qT_s[:, cc, qc * 512:(qc + 1) * 512],
                    start=(cc == 0), stop=(cc == 3))
        nc.vector.tensor_scalar_add(QT_s[:, dt, :], st, bq_s[:, dt, :])

    def emit_vproj(g):  # k tiles 2g, 2g+1
        st = s_pool.tile([128, 1024], F32, tag="s", name="st_v")
        for sub in range(2):
            kt = g * 2 + sub
            sl = st[:, sub * 512:(sub + 1) * 512]
            for cc in range(4):
                nc.tensor.matmul(
                    sl,
                    lhsT=vT_s[:, cc, kt * 128:(kt + 1) * 128],
                    rhs=wv_s[:, cc, :],
                    start=(cc == 0), stop=False)
            nc.tensor.matmul(sl, lhsT=ones[:, 0:128], rhs=bv_s,
                             start=False, stop=True)
            nc.vector.tensor_copy(
                out=V_s[:, kt, :, 0:64],
                in_=sl.rearrange("p (h w) -> p h w", w=64))

    # ---- attention ----------------------------------------------------------
    def emit_head(h):
        dt, even = h // 2, (h % 2 == 0)
        ctx_ps = c_pool.tile([128, 1024], F32, tag="c", name="ctx_ps")
        kt_lhs = KT_s[(h % 2) * 64:(h % 2) * 64 + 64, dt, :]
        q_rhs = QT_s[(h % 2) * 64:(h % 2) * 64 + 64, dt, :]
        for kt in range(NKT):
            st = s_pool.tile([128, 1024], F32, tag="s", name="st_s")
            for qc in range(2):
                nc.tensor.matmul(
                    st[:, qc * 512:(qc + 1) * 512],
                    lhsT=kt_lhs[:, kt * 128:(kt + 1) * 128],
                    rhs=q_rhs[:, qc * 512:(qc + 1) * 512],
                    start=True, stop=True)
            pt = p_pool.tile([128, 1024], BF16, tag="p", name="pt")
            nc.scalar.activation(pt, st, Exp, scale=0.125)
            # [V|1] -> ctx rows 0-63, rowsum row 64
            for qc in range(2):
                nc.tensor.matmul(
                    ctx_ps[0:65, qc * 512:(qc + 1) * 512],
                    lhsT=V_s[:, kt, h, 0:65],
                    rhs=pt[:, qc * 512:(qc + 1) * 512],
                    start=(kt == 0), stop=(kt == NKT - 1))
        recip = post.tile([128, NQ], F32, tag="recip", name="recip")
        nc.vector.reciprocal(recip[64:65, :], ctx_ps[64:65, :])
        # broadcast across partitions via DRAM bounce (step-0 partition AP
        # is only legal on DRAM sources)
        dr = dram.tile([1, NQ], F32, tag="dr", name="dr")
        nc.sync.dma_start(out=dr, in_=recip[64:65, :])
        bc = post.tile([128, NQ], F32, tag="bc", name="bc")
        nc.sync.dma_start(out=bc[0:64, :], in_=dr.to_broadcast((64, NQ)))
        if even:
            nc.vector.tensor_mul(ctxn_s[0:64, dt, :],
                                 ctx_ps[0:64, :], bc[0:64, :])
        else:
            tmp = post.tile([64, NQ], BF16, tag="tmp", name="tmp")
            nc.vector.tensor_mul(tmp, ctx_ps[0:64, :], bc[0:64, :])
            # partition shift 0-63 -> 64-127 via SBUF->SBUF DMA
            nc.sync.dma_start(out=ctxn_s[64:128, dt, :], in_=tmp)
        if dbg is not None and h == 7:
            d_c7t = post.tile([128, NQ], F32, tag="dbgc", name="d_c7t")
            nc.vector.tensor_copy(out=d_c7t, in_=ctx_ps)
            nc.sync.dma_start(out=dbg[4][:], in_=d_c7t)
            nc.sync.dma_start(out=dbg[5][:], in_=bc[0:64, :])

    # ---- output projection --------------------------------------------------
    def emit_outproj(g):  # n tiles 2g, 2g+1
        st = c_pool.tile([128, 1024], F32, tag="c", name="st_o")
        for sub in range(2):
            nt = g * 2 + sub
            sl = st[:, sub * 512:(sub + 1) * 512]
            for dc in range(4):
                nc.tensor.matmul(
                    sl,
                    lhsT=ctxn_s[:, dc, nt * 128:(nt + 1) * 128],
                    rhs=wo_s[:, dc, :],
                    start=(dc == 0), stop=False)
            nc.tensor.matmul(sl, lhsT=ones[:, 0:128], rhs=bo_s,
                             start=False, stop=True)
            ot = outs.tile([128, D], F32, tag="o", name="ot")
            nc.vector.tensor_copy(out=ot, in_=sl)
            nc.sync.dma_start(out=out[nt * 128:(nt + 1) * 128, :], in_=ot)

    # ---- schedule -----------------------------------------------------------
    emit_kproj(0)
    emit_qproj(0)
    for g in range(8):
        emit_vproj(g)
    for h in range(H):
        emit_head(h)
        if h % 2 == 1 and h < 7:  # prefetch next d-tile's projections
            emit_kproj(h // 2 + 1)
            emit_qproj(h // 2 + 1)
    for g in range(4):
        emit_outproj(g)

    if dbg is not None:
        d_kt, d_qt, d_v, d_cx = dbg[0], dbg[1], dbg[2], dbg[3]
        nc.sync.dma_start(out=d_kt[:].rearrange("p (c n) -> p c n", c=4), in_=KT_s)
        nc.sync.dma_start(out=d_qt[:].rearrange("p (c n) -> p c n", c=4), in_=QT_s)
        nc.sync.dma_start(
            out=d_v[:].rearrange("p (k h w) -> p k h w", k=NKT, h=H), in_=V_s)
        nc.sync.dma_start(out=d_cx[:].rearrange("p (c n) -> p c n", c=4), in_=ctxn_s)


_NC_CACHE = None


def _get_nc():
    global _NC_CACHE
    if _NC_CACHE is None:
        _NC_CACHE = build_nc()
    return _NC_CACHE


def make_in_maps(query, key, value, Wq, bq, Wk, bk, Wv, bv, Wo, bo):
    bf = ml_dtypes.bfloat16
    f = np.float32
    query = np.asarray(query, f)
    key = np.asarray(key, f)
    value = np.asarray(value, f)
    shared = {
        "wq": np.asarray(Wq, f).astype(bf),
        "wk": np.asarray(Wk, f).astype(bf),
        "wv": np.asarray(Wv, f).astype(bf),
        "wo": np.asarray(Wo, f).astype(bf),
        "bq": np.asarray(bq, f).reshape(D, 1),
        "bk": np.asarray(bk, f).reshape(D, 1),
        "bv": np.asarray(bv, f).astype(bf).reshape(1, D),
        "bo": np.asarray(bo, f).astype(bf).reshape(1, D),
    }
    kTs = [np.ascontiguousarray(key[b].T).astype(bf) for b in range(B)]
    vTs = [np.ascontiguousarray(value[b].T).astype(bf) for b in range(B)]
    in_maps = []
    for c in range(8):
        b, half = c // 2, c % 2
        m = dict(shared)
        m["qT"] = np.ascontiguousarray(
            query[b, half * NQ:(half + 1) * NQ, :].T).astype(bf)
        m["kT"] = kTs[b]
        m["vT"] = vTs[b]
        in_maps.append(m)
    return in_maps


def run(inputs, trace=False):
    nc = _get_nc()
    in_maps = make_in_maps(**inputs)
    res = run_bass_kernel_spmd(nc, in_maps, core_ids=list(range(8)), trace=trace)
    out = np.empty((B, N, D), np.float32)
    for c in range(8):
        b, half = c // 2, c % 2
        out[b, half * NQ:(half + 1) * NQ, :] = res.results[c]["out"]
    return out, res


def kernel(**inputs):
    out, _ = run(inputs, trace=False)
    return out



# revision 10
# speedup vs baseline: 1.1378x; 1.1378x over previous
# Multi-head attention layer on 8 TRN2 NeuronCores (SPMD, no collectives).
#
# Problem: B=4, N=2048, D=512, H=8 heads (DK=64).
#   out = softmax((q@Wq+bq)(k@Wk+bk)^T / 8) (v@Wv+bv) @ Wo + bo   per (batch, head)
#
# Sharding: core c handles batch b=c//2 and query-row half c%2 (1024 rows).
# K/V projections are recomputed by both cores of a pair (cheap) so there is
# no cross-core communication at all.
#
# Per-core dataflow (all layouts chosen so NO on-chip transposes are needed;
# the host pre-transposes inputs to (channel, token) layout and casts bf16):
#   K^T(d,k)  = Wk-chunks.T @ kT          (PE), +bias via DVE copy
#   Q^T(d,q)  = Wq-chunks.T @ qT          (PE), +bias via DVE copy
#   V(k,d)    = vT-chunks.T @ Wv          (PE), +bias via DVE add of a
#               DMA-broadcast bias tile, stored per-head as [V(64)|1|pad]
#               blocks (pitch 66)
#   S^T(k,q)  = K^T_tile.T @ Q^T  per head  (PE, contraction d=64)
#   P^T       = exp(S^T/8)                (ACT, scale folded into activation)
#   ctx^T(d,q)= [V|1]-block.T @ P^T       (PE, accumulated over k; the ones
#               column makes the softmax denominator land in row 64)
#   norm      = ctx^T * broadcast(1/rowsum)  (DMA bcast of denom + DVE
#               reciprocal_approx_fast on the 64-partition tile)
#   out(n,d)  = ctxn-chunks.T @ Wo        (PE), +bo via DVE add
#
# The S -> exp -> AV chain is software-pipelined: S(kt+1) is emitted BEFORE
# AV(kt) so the PE never idles waiting for the scalar engine's exp, keeping
# the PE HAM clock gate warm (2.4 GHz).
from contextlib import ExitStack

import numpy as np
import ml_dtypes

import concourse.bass as bass
import concourse.mybir as mybir
import concourse.tile as tile
from concourse import bacc
from concourse.bass_utils import run_bass_kernel_spmd

BF16 = mybir.dt.bfloat16
F32 = mybir.dt.float32
Exp = mybir.ActivationFunctionType.Exp

B, N, D, H = 4, 2048, 512, 8
DK = D // H          # 64
NQ = N // 2          # 1024 query rows per core
HP = 66              # per-head pitch in V_s: [V(64)|1|pad]
NKT = N // 128       # 16 k tiles


def build_nc():
    nc = bacc.Bacc("TRN2", target_bir_lowering=False)

    qT = nc.dram_tensor("qT", (D, NQ), BF16, kind="ExternalInput")
    kT = nc.dram_tensor("kT", (D, N), BF16, kind="ExternalInput")
    vT = nc.dram_tensor("vT", (D, N), BF16, kind="ExternalInput")
    wq = nc.dram_tensor("wq", (D, D), BF16, kind="ExternalInput")
    wk = nc.dram_tensor("wk", (D, D), BF16, kind="ExternalInput")
    wv = nc.dram_tensor("wv", (D, D), BF16, kind="ExternalInput")
    wo = nc.dram_tensor("wo", (D, D), BF16, kind="ExternalInput")
    bq = nc.dram_tensor("bq", (D, 1), F32, kind="ExternalInput")
    bk = nc.dram_tensor("bk", (D, 1), F32, kind="ExternalInput")
    bv = nc.dram_tensor("bv", (1, D), BF16, kind="ExternalInput")
    bo = nc.dram_tensor("bo", (1, D), BF16, kind="ExternalInput")
    out = nc.dram_tensor("out", (NQ, D), F32, kind="ExternalOutput")

    with tile.TileContext(nc) as tc:
        with ExitStack() as ctx:
            emit(ctx, tc, qT, kT, vT, wq, wk, wv, wo, bq, bk, bv, bo, out)
    nc.compile()
    return nc


def emit(ctx, tc, qT, kT, vT, wq, wk, wv, wo, bq, bk, bv, bo, out):
    nc = tc.nc
    consts = ctx.enter_context(tc.tile_pool(name="consts", bufs=1))
    p_pool = ctx.enter_context(tc.tile_pool(name="p_pool", bufs=6))
    post = ctx.enter_context(tc.tile_pool(name="post", bufs=2))
    outs = ctx.enter_context(tc.tile_pool(name="outs", bufs=3))
    s_pool = ctx.enter_context(tc.tile_pool(name="s_pool", bufs=2, space="PSUM"))
    c_pool = ctx.enter_context(tc.tile_pool(name="c_pool", bufs=2, space="PSUM"))

    # ---- inputs (DMA order = first-use order; big tensors in halves) -------
    def load(name, shape, dt_, src_ap):
        t = consts.tile(shape, dt_, name=name)
        nc.sync.dma_start(out=t, in_=src_ap)
        return t

    def load_halves(name, shape, dt_, dram_t, n):
        t = consts.tile(shape, dt_, name=name)
        h = n // 2
        for i in range(2):
            nc.sync.dma_start(
                out=t[:, :, i * h:(i + 1) * h],
                in_=dram_t[:, i * h:(i + 1) * h].rearrange(
                    "(c p) n -> p c n", p=128))
        return t

    wk_s = load("wk_s", [128, 4, D], BF16, wk[:].rearrange("(c p) d -> p c d", p=128))
    kT_s = load_halves("kT_s", [128, 4, N], BF16, kT, N)
    wv_s = load("wv_s", [128, 4, D], BF16, wv[:].rearrange("(c p) d -> p c d", p=128))
    vT_s = load_halves("vT_s", [128, 4, N], BF16, vT, N)
    bv_bc = load("bv_bc", [128, D], BF16, bv[:].to_broadcast((128, D)))
    wq_s = load("wq_s", [128, 4, D], BF16, wq[:].rearrange("(c p) d -> p c d", p=128))
    qT_s = load_halves("qT_s", [128, 4, NQ], BF16, qT, NQ)
    bq_s = load("bq_s", [128, 4, 1], F32, bq[:].rearrange("(c p) o -> p c o", p=128))
    bk_s = load("bk_s", [128, 4, 1], F32, bk[:].rearrange("(c p) o -> p c o", p=128))
    wo_s = load("wo_s", [128, 4, D], BF16, wo[:].rearrange("(c p) d -> p c d", p=128))
    bo_bc = load("bo_bc", [128, D], BF16, bo[:].to_broadcast((128, D)))

    KT_s = consts.tile([128, 4, N], BF16)     # K^T, d on partitions
    QT_s = consts.tile([128, 4, NQ], BF16)    # Q^T, d on partitions
    # V with k on partitions; per (kt, head) a 128-wide stationary block:
    # even heads [V(64) | ones(64)], odd heads [ones(64) | V(64)].  The ones
    # half replicates the softmax denominator onto the 64 partitions opposite
    # the ctx rows, so normalization needs no partition broadcast.
    V_s = consts.tile([128, NKT, H, 128], BF16)
    ctxn_s = consts.tile([128, 4, NQ], BF16)  # normalized ctx^T, dmid on partitions

    V_pairs = V_s[:].rearrange("p t (j par) w -> p t par j w", par=2)
    nc.vector.memset(V_pairs[:, :, 0, :, 64:128], 1.0)  # even heads: ones right
    nc.vector.memset(V_pairs[:, :, 1, :, 0:64], 1.0)    # odd heads: ones left

    # ---- projections --------------------------------------------------------
    def emit_kproj(dt):
        for kh in range(2):
            st = s_pool.tile([128, 1024], F32, tag="s", name="st_k")
            for kc in range(2):
                for cc in range(4):
                    nc.tensor.matmul(
                        st[:, kc * 512:(kc + 1) * 512],
                        lhsT=wk_s[:, cc, dt * 128:(dt + 1) * 128],
                        rhs=kT_s[:, cc, kh * 1024 + kc * 512: kh * 1024 + (kc + 1) * 512],
                        start=(cc == 0), stop=(cc == 3))
            nc.vector.tensor_scalar_add(
                KT_s[:, dt, kh * 1024:(kh + 1) * 1024], st, bk_s[:, dt, :])

    def emit_qproj(dt):
        st = c_pool.tile([128, 1024], F32, tag="c", name="st_q")
        for qc in range(2):
            for cc in range(4):
                nc.tensor.matmul(
                    st[:, qc * 512:(qc + 1) * 512],
                    lhsT=wq_s[:, cc, dt * 128:(dt + 1) * 128],
                    rhs=qT_s[:, cc, qc * 512:(qc + 1) * 512],
                    start=(cc == 0), stop=(cc == 3))
        nc.vector.tensor_scalar_add(QT_s[:, dt, :], st, bq_s[:, dt, :])

    def emit_vproj(g):  # k tiles 2g, 2g+1
        st = s_pool.tile([128, 1024], F32, tag="s", name="st_v")
        for sub in range(2):
            kt = g * 2 + sub
            sl = st[:, sub * 512:(sub + 1) * 512]
            for cc in range(4):
                nc.tensor.matmul(
                    sl,
                    lhsT=vT_s[:, cc, kt * 128:(kt + 1) * 128],
                    rhs=wv_s[:, cc, :],
                    start=(cc == 0), stop=(cc == 3))
            # scatter per-head 64-col blocks: even heads to cols 0:64 of
            # their V_s slot, odd heads to cols 64:128
            sl_pairs = sl.rearrange("p (j par w) -> p par j w", par=2, w=64)
            bv_pairs = bv_bc[:].rearrange("p (j par w) -> p par j w", par=2, w=64)
            vt_pairs = V_s[:, kt].rearrange("p (j par) w -> p par j w", par=2)
            nc.vector.tensor_add(
                vt_pairs[:, 0, :, 0:64], sl_pairs[:, 0], bv_pairs[:, 0])
            nc.vector.tensor_add(
                vt_pairs[:, 1, :, 64:128], sl_pairs[:, 1], bv_pairs[:, 1])

    # ---- attention ----------------------------------------------------------
    def emit_s_exp(h, kt):
        """S^T tile for (head, kt) on PE, then exp on ACT. Returns pt."""
        dt = h // 2
        kt_lhs = KT_s[(h % 2) * 64:(h % 2) * 64 + 64, dt, :]
        q_rhs = QT_s[(h % 2) * 64:(h % 2) * 64 + 64, dt, :]
        st = s_pool.tile([128, 1024], F32, tag="s", name="st_s")
        for qc in range(2):
            nc.tensor.matmul(
                st[:, qc * 512:(qc + 1) * 512],
                lhsT=kt_lhs[:, kt * 128:(kt + 1) * 128],
                rhs=q_rhs[:, qc * 512:(qc + 1) * 512],
                start=True, stop=True)
        pt = p_pool.tile([128, 1024], BF16, tag="p", name="pt")
        nc.scalar.activation(pt, st, Exp, scale=0.125)
        return pt

    def emit_av(h, kt, pt, ctx_ps):
        # [V|ones] (even h) / [ones|V] (odd h): ctx rows on one 64-partition
        # half, the softmax denominator replicated on the other half
        for qc in range(2):
            nc.tensor.matmul(
                ctx_ps[:, qc * 512:(qc + 1) * 512],
                lhsT=V_s[:, kt, h, :],
                rhs=pt[:, qc * 512:(qc + 1) * 512],
                start=(kt == 0), stop=(kt == NKT - 1))

    def emit_head(h, interleave_vproj=False):
        dt, even = h // 2, (h % 2 == 0)
        ctx_ps = c_pool.tile([128, 1024], F32, tag="c", name="ctx_ps")
        # software pipeline: S(kt+1) is emitted before AV(kt) so the PE's
        # in-order queue never stalls waiting for exp(kt) on the ACT engine
        pt_prev = emit_s_exp(h, 0)
        for kt in range(1, NKT):
            if interleave_vproj and kt % 2 == 0:
                emit_vproj(kt // 2)
            pt = emit_s_exp(h, kt)
            emit_av(h, kt - 1, pt_prev, ctx_ps)
            pt_prev = pt
        emit_av(h, NKT - 1, pt_prev, ctx_ps)

        # normalize: ctx rows sit on one 64-partition half of ctx_ps, the
        # denominator (replicated 64x by the ones block) on the other half.
        # reciprocal_approx_fast only works at base partition 0 and cannot
        # read PSUM, so: DVE-copy the denominator to SBUF (same lanes), get
        # it onto partitions 0-63 (for even heads via the partition-shifting
        # SBUF->SBUF DMA), take the reciprocal there, and shift the result
        # onto the ctx lanes if needed.  One DVE multiply finishes.
        cl, dl = (0, 64) if even else (64, 0)   # ctx / denom partition bases
        den = post.tile([128, NQ], F32, tag="den", name="den")
        nc.vector.tensor_copy(out=den[dl:dl + 64, :], in_=ctx_ps[dl:dl + 64, :])
        if dl != 0:
            den2 = post.tile([128, NQ], F32, tag="den2", name="den2")
            nc.sync.dma_start(out=den2[0:64, :], in_=den[dl:dl + 64, :])
            den = den2
        rc = post.tile([128, NQ], F32, tag="rc", name="rc")
        nc.vector.reciprocal_approx_fast(out=rc[0:64, :], in_=den[0:64, :])
        if cl != 0:
            rc2 = post.tile([128, NQ], F32, tag="rc2", name="rc2")
            nc.sync.dma_start(out=rc2[cl:cl + 64, :], in_=rc[0:64, :])
            rc = rc2
        nc.vector.tensor_mul(ctxn_s[cl:cl + 64, dt, :],
                             ctx_ps[cl:cl + 64, :], rc[cl:cl + 64, :])

    # ---- output projection --------------------------------------------------
    def emit_outproj(g):  # n tiles 2g, 2g+1
        st = c_pool.tile([128, 1024], F32, tag="c", name="st_o")
        for sub in range(2):
            nt = g * 2 + sub
            sl = st[:, sub * 512:(sub + 1) * 512]
            for dc in range(4):
                nc.tensor.matmul(
                    sl,
                    lhsT=ctxn_s[:, dc, nt * 128:(nt + 1) * 128],
                    rhs=wo_s[:, dc, :],
                    start=(dc == 0), stop=(dc == 3))
            ot = outs.tile([128, D], F32, tag="o", name="ot")
            nc.vector.tensor_add(ot, sl, bo_bc)
            nc.sync.dma_start(out=out[nt * 128:(nt + 1) * 128, :], in_=ot)

    # ---- schedule -----------------------------------------------------------
    emit_kproj(0)
    emit_vproj(0)
    emit_qproj(0)
    for h in range(H):
        emit_head(h, interleave_vproj=(h == 0))
        if h % 2 == 1 and h < 7:  # prefetch next d-tile's projections
            emit_kproj(h // 2 + 1)
            emit_qproj(h // 2 + 1)
    for g in range(4):
        emit_outproj(g)


_NC_CACHE = None


def _get_nc():
    global _NC_CACHE
    if _NC_CACHE is None:
        _NC_CACHE = build_nc()
    return _NC_CACHE


def make_in_maps(query, key, value, Wq, bq, Wk, bk, Wv, bv, Wo, bo):
    bf = ml_dtypes.bfloat16
    f = np.float32
    query = np.asarray(query, f)
    key = np.asarray(key, f)
    value = np.asarray(value, f)
    shared = {
        "wq": np.asarray(Wq, f).astype(bf),
        "wk": np.asarray(Wk, f).astype(bf),
        "wv": np.asarray(Wv, f).astype(bf),
        "wo": np.asarray(Wo, f).astype(bf),
        "bq": np.asarray(bq, f).reshape(D, 1),
        "bk": np.asarray(bk, f).reshape(D, 1),
        "bv": np.asarray(bv, f).astype(bf).reshape(1, D),
        "bo": np.asarray(bo, f).astype(bf).reshape(1, D),
    }
    kTs = [np.ascontiguousarray(key[b].T).astype(bf) for b in range(B)]
    vTs = [np.ascontiguousarray(value[b].T).astype(bf) for b in range(B)]
    in_maps = []
    for c in range(8):
        b, half = c // 2, c % 2
        m = dict(shared)
        m["qT"] = np.ascontiguousarray(
            query[b, half * NQ:(half + 1) * NQ, :].T).astype(bf)
        m["kT"] = kTs[b]
        m["vT"] = vTs[b]
        in_maps.append(m)
    return in_maps


def run(inputs, trace=False):
    nc = _get_nc()
    in_maps = make_in_maps(**inputs)
    res = run_bass_kernel_spmd(nc, in_maps, core_ids=list(range(8)), trace=trace)
    out = np.empty((B, N, D), np.float32)
    for c in range(8):
        b, half = c // 2, c % 2
        out[b, half * NQ:(half + 1) * NQ, :] = res.results[c]["out"]
    return out, res


def kernel(**inputs):
    out, _ = run(inputs, trace=False)
    return out
